# revision 1
# baseline (speedup 1.0000x reference)
import numpy as np
import jax
import jax.numpy as jnp
from functools import partial

# nn_GCN_15333033247254 — hardcoded problem shapes
N = 100000      # nodes
P = 8           # cores
NP_PER = N // P # 12500 nodes per core
F_IN, H, C = 128, 128, 8


def _build_graph_np(edge_index):
    # self-loops (PyG gcn_norm default)
    loop = np.arange(N, dtype=np.int64)
    src = np.concatenate([np.asarray(edge_index[0]), loop])
    dst = np.concatenate([np.asarray(edge_index[1]), loop])
    deg = np.bincount(dst, minlength=N).astype(np.float32)
    dis = np.where(deg > 0, 1.0 / np.sqrt(np.maximum(deg, 1.0)), 0.0).astype(np.float32)
    norm = dis[src] * dis[dst]
    return src, dst, norm


def _partition_edges(src, dst, norm):
    # Shard edges by destination-node bucket (device p owns dst rows
    # [p*NP_PER, (p+1)*NP_PER)); pad buckets to equal length with
    # norm=0 edges so padded messages contribute nothing.
    bucket = dst // NP_PER
    order = np.argsort(bucket, kind="stable")
    src_s, dst_s, norm_s = src[order], dst[order], norm[order]
    counts = np.bincount(bucket, minlength=P)
    e_pad = int(counts.max())
    src_p = np.zeros((P, e_pad), dtype=np.int32)
    dstl_p = np.zeros((P, e_pad), dtype=np.int32)
    norm_p = np.zeros((P, e_pad), dtype=np.float32)
    off = 0
    for p in range(P):
        c = int(counts[p])
        src_p[p, :c] = src_s[off:off + c]
        dstl_p[p, :c] = dst_s[off:off + c] - p * NP_PER
        norm_p[p, :c] = norm_s[off:off + c]
        off += c
    return src_p, dstl_p, norm_p


def _gcn_sharded(x_full, src_e, dstl_e, norm_e, W1, b1, W2, b2):
    # Runs per-device under pmap. x/W/b replicated; edges sharded by dst.
    xw = x_full @ W1                                   # [N, H] replicated
    msgs = xw[src_e] * norm_e[:, None]                 # gather + scale
    h = jax.ops.segment_sum(msgs, dstl_e, num_segments=NP_PER) + b1
    h = jax.nn.relu(h)                                 # [NP_PER, H] local rows
    # halo exchange: every device needs all rows of h for layer-2 gather
    h_full = jax.lax.all_gather(h, "i", axis=0).reshape(N, H)
    hw = h_full @ W2                                   # [N, C]
    msgs2 = hw[src_e] * norm_e[:, None]
    o = jax.ops.segment_sum(msgs2, dstl_e, num_segments=NP_PER) + b2
    return jax.nn.log_softmax(o, axis=1)               # [NP_PER, C]


def _run_on_devices(devs, x, src_p, dstl_p, norm_p, W1, b1, W2, b2):
    f = jax.pmap(
        partial(_gcn_sharded,
                W1=jnp.asarray(W1), b1=jnp.asarray(b1),
                W2=jnp.asarray(W2), b2=jnp.asarray(b2)),
        axis_name="i",
        in_axes=(None, 0, 0, 0),
        devices=devs,
    )
    out = f(jnp.asarray(x), src_p, dstl_p, norm_p)     # [P, NP_PER, C]
    return np.asarray(out).reshape(N, C).astype(np.float32)


def kernel(x, edge_index, W1, b1, W2, b2):
    x = np.asarray(x, dtype=np.float32)
    src, dst, norm = _build_graph_np(edge_index)
    src_p, dstl_p, norm_p = _partition_edges(src, dst, norm)

    try:
        devs = jax.devices()[:P]
        if len(devs) < P:
            raise RuntimeError("fewer than 8 devices")
        return _run_on_devices(devs, x, src_p, dstl_p, norm_p, W1, b1, W2, b2)
    except Exception:
        # CPU fallback: same math, single device
        xw = x @ np.asarray(W1)
        msgs = xw[src] * norm[:, None]
        h = np.zeros((N, H), dtype=np.float32)
        np.add.at(h, dst, msgs)
        h = np.maximum(h + np.asarray(b1), 0.0)
        hw = h @ np.asarray(W2)
        msgs2 = hw[src] * norm[:, None]
        o = np.zeros((N, C), dtype=np.float32)
        np.add.at(o, dst, msgs2)
        o = o + np.asarray(b2)
        m = o.max(axis=1, keepdims=True)
        lse = np.log(np.exp(o - m).sum(axis=1, keepdims=True)) + m
        return (o - lse).astype(np.float32)



# revision 2
# speedup vs baseline: 3.9970x; 3.9970x over previous
import numpy as np

# nn_GCN_15333033247254 — hardcoded problem shapes
N = 100000
P = 8
F_IN, H, C = 128, 128, 8

_cache = {}


def _bass_path(x, edge_index, W1, b1, W2, b2):
    from gcn_bass import Cfg, host_prep, build_nc, run
    cfg = Cfg()
    prep = host_prep(cfg, edge_index)
    key = ("nc", prep["E_pad"], prep["P_tc"].tobytes())
    nc = _cache.get(key)
    if nc is None:
        nc = build_nc(cfg, prep["P_tc"], prep["E_pad"])
        _cache.clear()
        _cache[key] = nc
    inputs = dict(x=x, W1=W1, b1=b1, W2=W2, b2=b2)
    return run(cfg, inputs, prep, nc)


def _cpu_fallback(x, edge_index, W1, b1, W2, b2):
    loop = np.arange(N, dtype=np.int64)
    src = np.concatenate([np.asarray(edge_index[0]), loop])
    dst = np.concatenate([np.asarray(edge_index[1]), loop])
    deg = np.bincount(dst, minlength=N).astype(np.float32)
    dis = np.where(deg > 0, 1.0 / np.sqrt(np.maximum(deg, 1.0)), 0.0)
    norm = (dis[src] * dis[dst]).astype(np.float32)
    xw = x @ np.asarray(W1)
    msgs = xw[src] * norm[:, None]
    h = np.zeros((N, H), dtype=np.float32)
    np.add.at(h, dst, msgs)
    h = np.maximum(h + np.asarray(b1), 0.0)
    hw = h @ np.asarray(W2)
    msgs2 = hw[src] * norm[:, None]
    o = np.zeros((N, C), dtype=np.float32)
    np.add.at(o, dst, msgs2)
    o = o + np.asarray(b2)
    m = o.max(axis=1, keepdims=True)
    lse = np.log(np.exp(o - m).sum(axis=1, keepdims=True)) + m
    return (o - lse).astype(np.float32)


def kernel(x, edge_index, W1, b1, W2, b2):
    x = np.asarray(x, dtype=np.float32)
    edge_index = np.asarray(edge_index)
    try:
        return _bass_path(x, edge_index, W1, b1, W2, b2)
    except Exception:
        import traceback
        traceback.print_exc()
        return _cpu_fallback(x, edge_index, W1, b1, W2, b2)
